# revision 48
# baseline (speedup 1.0000x reference)
"""Multi-head attention (B=2, T=2048, D=1024, R=16 heads, v=k) on 8 TRN2 cores.

Sharding: batch*heads across cores. Core c handles batch c//4, heads
[4*(c%4), 4*(c%4)+4). Each core computes its 4 heads' attention plus the
partial output projection; the host sums the 4 partials per batch.

All matmul operands are bf16 (FWL weight loads, halved DMA/SBUF); exp
stays exact on the Scalar engine (128 ACTs over [128,1024] psum tiles)
and paces the steady state.  Projections for the second head-pair, the
V projection, and the trailing output projection are interleaved into
the attention segments as PE fillers so the Scalar engine streams
activations nearly continuously after a DMA-bound lead-in.

Per-core dataflow:
  khT/qhT [128, T] bf16 per head-pair p  (rows 0:64 head 2p, 64:128 head 2p+1)
  vh      [128, tb, j, 65] bf16          (64 V cols + ones col -> softmax denom)
  S^T     s_ps [128, 1024] fp32 psum     (two heads row-tiled)
  P^T = exp(S^T/8) -> bf16, one ACT per tile (Scalar)
  PV      M=65 per head into two psum banks (row 64 = denominator)
  bcast   one K=33 selector matmul broadcasts both heads' denominators
          into [128,512]; deferred past the next segment's first S pair
  merged  [128, T] bf16 per pair (h1 written with partition-shifted DVE mul)
  outproj 2 accumulating K=128 matmuls per [128,512] out tile; the last
          tq group is merged per t-block so the tail drains early
"""

import numpy as np

B, T, D = 2, 2048, 1024
R = 16
DH = 64
NCORES = 8
GROUPS = 4          # head groups (cores per batch)
HPG = 4             # heads per group/core
DG = HPG * DH       # 256 projected cols per core
NCHUNK = D // 128   # 8 contraction chunks
NTB = T // 128      # 16 t-blocks
NTQ = T // 512      # 4 tq tiles
VW = DH + 1         # 65: V columns + ones column

_CACHE = {}


def _build():
    import concourse.mybir as mybir
    import concourse.tile as tile
    from bass_rust import add_dep_helper
    from concourse import bacc

    FP32 = mybir.dt.float32
    FP32R = mybir.dt.float32r
    BF16 = mybir.dt.bfloat16
    EXP = mybir.ActivationFunctionType.Exp

    nc = bacc.Bacc("TRN2", target_bir_lowering=False, debug=False)

    qT = nc.dram_tensor("qT", [D, T], BF16, kind="ExternalInput")
    kT = nc.dram_tensor("kT", [D, T], BF16, kind="ExternalInput")
    wq = nc.dram_tensor("wq", [D, DG], BF16, kind="ExternalInput")
    wk = nc.dram_tensor("wk", [D, DG], BF16, kind="ExternalInput")
    wv = nc.dram_tensor("wv", [D, DG], BF16, kind="ExternalInput")
    wo = nc.dram_tensor("wo", [DG, D], BF16, kind="ExternalInput")
    out = nc.dram_tensor("out", [T, D], FP32, kind="ExternalOutput")

    with tile.TileContext(nc) as tc:
        with (
            tc.tile_pool(name="weights", bufs=1) as wpool,
            tc.tile_pool(name="persist", bufs=1) as pers,
            tc.tile_pool(name="stream", bufs=1) as stream,
            tc.tile_pool(name="pT", bufs=12) as ppool,
            tc.tile_pool(name="small", bufs=4) as small,
            tc.tile_pool(name="outstage", bufs=4) as ostage,
            tc.tile_pool(name="s_ps", bufs=2, space="PSUM") as sps,
            tc.tile_pool(name="pv_ps", bufs=2, space="PSUM") as pvps,
            tc.tile_pool(name="flex_ps", bufs=2, space="PSUM") as fps,
        ):
            # ---------- DMA: weights, then kT, then qT (tq0 halves first) ----
            wq_sb = wpool.tile([128, NCHUNK, DG], BF16)
            wk_sb = wpool.tile([128, NCHUNK, DG], BF16)
            wv_sb = wpool.tile([128, NCHUNK, DG], BF16)
            nc.sync.dma_start(wk_sb[:], wk[:].rearrange("(c p) d -> p c d", p=128))
            kch_all = stream.tile([128, NCHUNK, T], BF16, tag="kch")
            qch_all = stream.tile([128, NCHUNK, T], BF16, tag="qch")
            kch = [kch_all[:, c, :] for c in range(NCHUNK)]
            qch = [qch_all[:, c, :] for c in range(NCHUNK)]
            # kT in chunk pairs so the K projection paces with the transfer
            for cp in range(4):
                nc.sync.dma_start(
                    kch_all[:, 2 * cp : 2 * cp + 2, :],
                    kT[cp * 256 : (cp + 1) * 256, :].rearrange(
                        "(c p) t -> p c t", p=128
                    ),
                )
            nc.sync.dma_start(
                qch_all[:, :, 0:512],
                qT[:, 0:512].rearrange("(c p) t -> p c t", p=128),
            )
            nc.sync.dma_start(wq_sb[:], wq[:].rearrange("(c p) d -> p c d", p=128))
            nc.sync.dma_start(wv_sb[:], wv[:].rearrange("(c p) d -> p c d", p=128))
            wo_sb = []
            for p in range(2):
                t_ = wpool.tile([128, D], BF16, tag=f"wo{p}", name=f"wo{p}")
                nc.sync.dma_start(t_[:], wo[p * 128 : (p + 1) * 128, :])
                wo_sb.append(t_)
            nc.sync.dma_start(
                qch_all[:, :, 512:T],
                qT[:, 512:T].rearrange("(c p) t -> p c t", p=128),
            )

            # ---------- constants ----------
            ones_f32 = pers.tile([128, DH], FP32, tag="ones_f32")
            nc.gpsimd.memset(ones_f32[:], 1.0)
            ones_bf = pers.tile([128, DH], BF16, tag="ones_bf")
            nc.vector.tensor_copy(ones_bf[:], ones_f32[:])
            warm_sb = pers.tile([128, 512], BF16, tag="warm_sb")
            nc.gpsimd.memset(warm_sb[:], 0.5)
            sel_f = pers.tile([128, 128], FP32, tag="sel_f")
            nc.gpsimd.memset(sel_f[:], 0.0)
            nc.gpsimd.memset(sel_f[64:65, 0:64], 1.0)
            nc.gpsimd.memset(sel_f[96:97, 64:128], 1.0)
            sel = pers.tile([128, 128], FP32R, tag="sel")
            nc.vector.tensor_copy(sel[:], sel_f[:])
            zero_f32 = pers.tile([128, 512], FP32, tag="zero_f32")
            nc.gpsimd.memset(zero_f32[:], 0.0)
            sums_pair = pers.tile([128, 512], FP32R, tag="sums_pair")
            nc.vector.tensor_copy(sums_pair[:], zero_f32[:])

            qhT = [pers.tile([128, T], BF16, tag=f"qhT{p}", name=f"qhT{p}") for p in range(2)]
            khT = [pers.tile([128, T], BF16, tag=f"khT{p}", name=f"khT{p}") for p in range(2)]
            vh = pers.tile([128, NTB, HPG, VW], BF16, tag="vh")
            nc.vector.tensor_copy(
                vh[:, :, :, DH],
                ones_bf[:].rearrange("p (a b) -> p a b", a=NTB),
            )
            merged = [pers.tile([128, T], BF16, tag=f"mg{p}", name=f"mg{p}") for p in range(2)]

            # ---------- PE warmup (HAM) ----------
            # enough dummy matmuls to keep the PE warm through the input DMA
            wacc = fps.tile([128, 512], FP32, tag="flex", name="warmacc")
            for i in range(10):
                nc.tensor.matmul(
                    wacc[:], warm_sb[:, 0:128], warm_sb[:],
                    start=True, stop=True,
                )

            # ---------- lead-in projections: khT-p0 (full), qhT-p0 tt0 ------
            # khT-p0 via the (still free) s_ps ring: two [128,1024] accs
            for half in range(2):
                acc = sps.tile([128, 1024], FP32, tag="s", name=f"kp0acc{half}")
                for tt in range(2):
                    col = slice(tt * 512, (tt + 1) * 512)
                    tcol = slice(half * 1024 + tt * 512, half * 1024 + (tt + 1) * 512)
                    for c in range(NCHUNK):
                        nc.tensor.matmul(
                            acc[:, col],
                            wk_sb[:, c, 0:128],
                            kch_all[:, c, tcol],
                            start=(c == 0),
                            stop=(c == NCHUNK - 1),
                        )
                nc.scalar.copy(
                    khT[0][:, half * 1024 : (half + 1) * 1024], acc[:]
                )
            def f_vh(tb, p):
                def emit():
                    vacc = fps.tile([128, 512], FP32, tag="flex", name=f"vacc{tb}_{p}")
                    for c in range(NCHUNK):
                        nc.tensor.matmul(
                            vacc[:, 0:128],
                            kch_all[:, c, tb * 128 : (tb + 1) * 128],
                            wv_sb[:, c, p * 128 : (p + 1) * 128],
                            start=(c == 0),
                            stop=(c == NCHUNK - 1),
                        )
                    nc.vector.tensor_copy(
                        vh[:, tb, 2 * p : 2 * p + 2, 0:DH],
                        vacc[:, 0:128].rearrange("p (j d) -> p j d", j=2),
                    )
                return emit

            qacc0 = fps.tile([128, 512], FP32, tag="flex", name="qp0acc0")
            for c in range(NCHUNK):
                nc.tensor.matmul(
                    qacc0[:],
                    wq_sb[:, c, 0:128],
                    qch_all[:, c, 0:512],
                    start=(c == 0),
                    stop=(c == NCHUNK - 1),
                )
            nc.scalar.copy(qhT[0][:, 0:512], qacc0[:])

            # ---------- filler emitters (run inside attention segments) -----
            def f_proj_halves(which, p, tt):
                # split the 8-chunk accumulation into two fillers so each
                # PE burst between S matmuls stays short
                w_sb = wk_sb if which == "k" else wq_sb
                dstl = khT if which == "k" else qhT
                ch = kch_all if which == "k" else qch_all
                holder = {}

                def emit_a():
                    holder["acc"] = fps.tile(
                        [128, 512], FP32, tag="flex", name=f"{which}acc{p}_{tt}"
                    )
                    for c in range(4):
                        nc.tensor.matmul(
                            holder["acc"][:],
                            w_sb[:, c, p * 128 : (p + 1) * 128],
                            ch[:, c, tt * 512 : (tt + 1) * 512],
                            start=(c == 0),
                            stop=False,
                        )

                def emit_b():
                    acc = holder["acc"]
                    for c in range(4, NCHUNK):
                        nc.tensor.matmul(
                            acc[:],
                            w_sb[:, c, p * 128 : (p + 1) * 128],
                            ch[:, c, tt * 512 : (tt + 1) * 512],
                            start=False,
                            stop=(c == NCHUNK - 1),
                        )
                    nc.vector.tensor_copy(
                        dstl[p][:, tt * 512 : (tt + 1) * 512], acc[:]
                    )
                return [emit_a, emit_b]

            def f_proj(which, p, tt):
                return f_proj_halves(which, p, tt)

            def f_outproj(otq, scalar_drain=False):
                def emit_for(tb, nt):
                    def emit():
                        tbs = slice(tb * 128, (tb + 1) * 128)
                        op = fps.tile([128, 512], FP32, tag="flex", name=f"op{tb}_{nt}")
                        for p in range(2):
                            nc.tensor.matmul(
                                op[:],
                                merged[p][:, tbs],
                                wo_sb[p][:, nt * 512 : (nt + 1) * 512],
                                start=(p == 0),
                                stop=(p == 1),
                            )
                        ob = ostage.tile([128, 512], FP32, tag="ob")
                        if scalar_drain:
                            nc.scalar.copy(ob[:], op[:])
                        else:
                            nc.vector.tensor_copy(ob[:], op[:])
                        nc.sync.dma_start(out[tbs, nt * 512 : (nt + 1) * 512], ob[:])
                    return emit
                return [
                    emit_for(tb, nt)
                    for tb in range(otq * 4, (otq + 1) * 4)
                    for nt in range(2)
                ]

            # segment order and their PE fillers (each list spread over 16 tk
            # slots; a filler is ~0.4-1.7us of PE work)
            segments = [
                ((0, 0), [f_vh(tb, 0) for tb in range(NTB)]
                         + f_proj("q", 0, 1) + f_proj("q", 1, 0)),
                ((1, 0), [f for tt in range(4) for f in f_proj("k", 1, tt)]),
                ((0, 1), [f_vh(tb, 1) for tb in range(NTB)]
                         + f_proj("q", 1, 1)),
                ((1, 1), f_outproj(0) + f_proj("q", 0, 2)),
                ((2, 0), f_outproj(1) + f_proj("q", 0, 3)),
                ((3, 0), f_proj("q", 1, 2)),
                ((2, 1), f_proj("q", 1, 3)),
                ((3, 1), f_outproj(2)),
            ]

            pending_bcasts = []
            for (tq, p), fillers in segments:
                tqs = slice(tq * 512, (tq + 1) * 512)
                # spread fillers over the 16 tk slots
                fill_at = {}
                nf = len(fillers)
                for i, f in enumerate(fillers):
                    slot = min(NTB - 1, (i * NTB) // max(nf, 1))
                    fill_at.setdefault(slot, []).append(f)

                pv = [
                    pvps.tile([128, 512], FP32, tag="pv", name=f"pv{h}")
                    for h in range(2)
                ]
                pTs = [None] * NTB

                def do_pv(tk, after=None):
                    for h in range(2):
                        m = nc.tensor.matmul(
                            pv[h][0:VW, :],
                            vh[:, tk, 2 * p + h, :],
                            pTs[tk][:, h * 512 : (h + 1) * 512],
                            start=(tk == 0),
                            stop=(tk == NTB - 1),
                        )
                        if after is not None:
                            add_dep_helper(
                                m.ins, after.ins,
                                reason="PV ordered after next S pair",
                            )

                for tk in range(NTB):
                    s_ps = sps.tile([128, 1024], FP32, tag="s")
                    sB = None
                    for h in range(2):
                        lo, hi = h * 64, (h + 1) * 64
                        sB = nc.tensor.matmul(
                            s_ps[:, h * 512 : (h + 1) * 512],
                            khT[p][lo:hi, tk * 128 : (tk + 1) * 128],
                            qhT[p][lo:hi, tqs],
                            start=True,
                            stop=True,
                        )
                    pTs[tk] = ppool.tile([128, 1024], BF16, tag="pT", name=f"pT{tk}")
                    nc.scalar.activation(pTs[tk][:], s_ps[:], EXP, scale=0.125)
                    if tk == 1 and pending_bcasts:
                        for bi in pending_bcasts:
                            add_dep_helper(
                                bi.ins, sB.ins,
                                reason="bcast after next segment start",
                            )
                        pending_bcasts = []
                    for f in fill_at.get(tk, []):
                        f()
                    if tk >= 2:
                        do_pv(tk - 2, after=sB)
                do_pv(NTB - 2)
                do_pv(NTB - 1)

                # ---- softmax denominators + merge ----
                nc.vector.tensor_copy(sums_pair[64:65, :], pv[0][64:65, :])
                nc.vector.tensor_copy(sums_pair[96:97, :], pv[1][64:65, :])
                bc = fps.tile([128, 512], FP32, tag="flex", name="bc")
                bm = nc.tensor.matmul(
                    bc[:],
                    sel[64:97, :],
                    sums_pair[64:97, :],
                    start=True,
                    stop=True,
                )
                pending_bcasts.append(bm)
                rec = small.tile([128, 512], FP32, tag="rec")
                nc.vector.reciprocal_approx_fast(rec[:], bc[:])
                if (tq, p) != (3, 1):
                    for h in range(2):
                        nc.vector.tensor_mul(
                            merged[p][h * 64 : (h + 1) * 64, tqs],
                            pv[h][0:DH, :],
                            rec[h * 64 : (h + 1) * 64, :],
                        )
                else:
                    # final segment: merge per t-block and emit the tail
                    # output projection as soon as each block is ready
                    tail_ops = f_outproj(3, scalar_drain=True)
                    for cb in range(4):
                        cs = slice(cb * 128, (cb + 1) * 128)
                        for h in range(2):
                            nc.vector.tensor_mul(
                                merged[p][h * 64 : (h + 1) * 64,
                                          tq * 512 + cb * 128 : tq * 512 + (cb + 1) * 128],
                                pv[h][0:DH, cs],
                                rec[h * 64 : (h + 1) * 64, cs],
                            )
                        tail_ops[2 * cb]()
                        tail_ops[2 * cb + 1]()



    nc.compile()
    return nc


def _get_nc():
    if "nc" not in _CACHE:
        _CACHE["nc"] = _build()
    return _CACHE["nc"]


def kernel(q, k, q_map, k_map, v_map, output_map, trace=False):
    import ml_dtypes
    from concourse.bass_utils import run_bass_kernel_spmd

    bf = ml_dtypes.bfloat16
    q = np.asarray(q, dtype=np.float32)
    k = np.asarray(k, dtype=np.float32)
    q_map = np.asarray(q_map, dtype=np.float32)
    k_map = np.asarray(k_map, dtype=np.float32)
    v_map = np.asarray(v_map, dtype=np.float32)
    output_map = np.asarray(output_map, dtype=np.float32)

    nc = _get_nc()

    qTs = [np.ascontiguousarray(q[b].T).astype(bf) for b in range(B)]
    kTs = [np.ascontiguousarray(k[b].T).astype(bf) for b in range(B)]
    in_maps = []
    for c in range(NCORES):
        b, g = c // GROUPS, c % GROUPS
        cs = slice(g * DG, (g + 1) * DG)
        in_maps.append(
            {
                "qT": qTs[b],
                "kT": kTs[b],
                "wq": np.ascontiguousarray(q_map[:, cs]).astype(bf),
                "wk": np.ascontiguousarray(k_map[:, cs]).astype(bf),
                "wv": np.ascontiguousarray(v_map[:, cs]).astype(bf),
                "wo": np.ascontiguousarray(output_map[cs, :]).astype(bf),
            }
        )

    res = run_bass_kernel_spmd(nc, in_maps, list(range(NCORES)), trace=trace)
    if trace:
        _CACHE["last_exec_time_ns"] = res.exec_time_ns
        _CACHE["last_results"] = res

    outp = np.empty((B, T, D), dtype=np.float32)
    for b in range(B):
        acc = res.results[b * GROUPS]["out"].astype(np.float32)
        for g in range(1, GROUPS):
            acc = acc + res.results[b * GROUPS + g]["out"]
        outp[b] = acc
    return outp
